# revision 1
# baseline (speedup 1.0000x reference)
"""Trainium2 Bass kernel for EnhancedMultiHeadSelfAttention (dense transformer block).

Sharding: sequence-parallel over 8 cores. Each core owns L/8 = 256 query rows.
LN1 + K/V projection for all 2048 tokens are replicated on every core (cheaper
than on-chip AllReduce at this size); scores/softmax/attn@V/out-proj/LN2/FFN are
computed only for the core's own 256 rows. No collectives.

Layout: activations are kept feature-major ("transposed", [feature, token]) so
every linear layer is matmul(out=[cols, tok], lhsT=W[k,cols], rhs=actT[k,tok])
with natural weight layout and no on-device transposes. All matmuls run as
float32r (full fp32 data, bf16-rate PE throughput for free dim >= 256).

Math notes:
 - clip(scores,-10,10) never binds: |cos|*0.125 + bias in [-0.125, 0.225].
 - softmax needs no max-subtraction for the same reason.
 - the query-side half of the lcc bias is a per-query constant factor in
   exp-space and cancels in softmax normalization; only the key-side half is
   applied (as per-partition ACT bias in the exp).
 - softmax denominators come from an appended ones-column in V.
 - LN gains/biases are folded into the following matmul's weights on the host.
"""

import numpy as np

import concourse.bass as bass
import concourse.tile as tile
from concourse import bacc, mybir
from concourse.bass_utils import run_bass_kernel_spmd

F32 = mybir.dt.float32
F32R = mybir.dt.float32r

L = 2048          # sequence length
D = 1024          # model dim
H = 16            # heads
DH = 64           # head dim
FF = 4096         # ffn hidden
P = 128           # partitions
NCORES = 8
LQ = L // NCORES  # 256 own query rows per core
DC = D // P       # 8 d-model chunks
FC = FF // P      # 32 ffn chunks
KC = L // P       # 16 key chunks
NBLK = 4          # token blocks of 512 for the replicated phase
BLK = L // NBLK   # 512

# CoreSim doesn't implement Gelu; test_sim swaps this to Identity and checks
# against a gelu-less reference. Hardware always uses the real (erf) Gelu.
GELU_FUNC = mybir.ActivationFunctionType.Gelu

LN_EPS = 1e-5
NORM_EPS = 1e-12
SCALING = DH ** -0.5
LCC = 0.1


def _mm(nc, out, lhsT, rhs, start, stop):
    assert lhsT.dtype == F32R and rhs.dtype == F32R, (lhsT.dtype, rhs.dtype)
    nc.tensor.matmul(out, lhsT, rhs, start=start, stop=stop)


def emit(tc):
    nc = tc.nc

    xt = nc.dram_tensor("xt", [D, L], F32R, kind="ExternalInput").ap()
    xot = nc.dram_tensor("xot", [D, LQ], F32R, kind="ExternalInput").ap()
    wq = nc.dram_tensor("wq", [D, D], F32R, kind="ExternalInput").ap()
    wk = nc.dram_tensor("wk", [D, D], F32R, kind="ExternalInput").ap()
    wv = nc.dram_tensor("wv", [D, D], F32R, kind="ExternalInput").ap()
    wo = nc.dram_tensor("wo", [D, D], F32R, kind="ExternalInput").ap()
    wf1 = nc.dram_tensor("wf1", [D, FF], F32R, kind="ExternalInput").ap()
    wf2 = nc.dram_tensor("wf2", [FF, D], F32R, kind="ExternalInput").ap()
    bq = nc.dram_tensor("bq", [P, DC], F32, kind="ExternalInput").ap()
    bk = nc.dram_tensor("bk", [P, DC], F32, kind="ExternalInput").ap()
    bv = nc.dram_tensor("bv", [D], F32, kind="ExternalInput").ap()
    bo = nc.dram_tensor("bo", [P, DC], F32, kind="ExternalInput").ap()
    bf1 = nc.dram_tensor("bf1", [P, FC], F32, kind="ExternalInput").ap()
    bf2 = nc.dram_tensor("bf2", [P, DC], F32, kind="ExternalInput").ap()
    lcck = nc.dram_tensor("lcck", [P, KC], F32, kind="ExternalInput").ap()
    selr = nc.dram_tensor("selr", [P, P], F32R, kind="ExternalInput").ap()
    selb = nc.dram_tensor("selb", [H, DC * P], F32R, kind="ExternalInput").ap()
    onesc = nc.dram_tensor("onesc", [P, 3], F32R, kind="ExternalInput").ap()
    ones1r = nc.dram_tensor("ones1r", [1, P], F32R, kind="ExternalInput").ap()
    vones = nc.dram_tensor("vones", [P, KC], F32R, kind="ExternalInput").ap()
    out_t = nc.dram_tensor("out_t", [D, LQ], F32, kind="ExternalOutput").ap()

    xt3 = xt.rearrange("(c p) t -> p c t", p=P)        # [128, 8, 2048]
    xot3 = xot.rearrange("(c p) t -> p c t", p=P)      # [128, 8, 256]
    wq3 = wq.rearrange("(c p) n -> p c n", p=P)        # [128, 8, 1024]
    wk3 = wk.rearrange("(c p) n -> p c n", p=P)
    wv3 = wv.rearrange("(c p) n -> p c n", p=P)
    wo3 = wo.rearrange("(c p) n -> p c n", p=P)
    wf13 = wf1.rearrange("(c p) n -> p c n", p=P)      # [128, 8, 4096]
    wf23 = wf2.rearrange("(c p) n -> p c n", p=P)      # [128, 32, 1024]
    out3 = out_t.rearrange("(c p) t -> p c t", p=P)    # [128, 8, 256]

    # ---- persistent small constants -------------------------------------
    singles = tc.alloc_tile_pool(name="singles", bufs=1)
    ones_1x128 = singles.tile([1, P], F32R)  # K=1 broadcast lhsT
    nc.sync.dma_start(ones_1x128, ones1r)
    onesc_sb = singles.tile([P, 3], F32R)
    nc.sync.dma_start(onesc_sb, onesc)
    ones_col = onesc_sb[:, 0:1]              # K=128 -> M=1 reduction lhsT
    # head-norm selectors (host-precomputed):
    # selr_sb[:, m, h] = 1 if head h belongs to chunk m at this partition;
    # selb_sb[h, m*128+p] = transpose, for broadcasting norms back to chunks
    selr_sb = singles.tile([P, DC, H], F32R)
    nc.sync.dma_start(selr_sb, selr.rearrange("p (m h) -> p m h", h=H))
    selb_sb = singles.tile([H, DC, P], F32R)
    nc.sync.dma_start(selb_sb, selb.rearrange("h (m p) -> h m p", p=P))
    vones_sb = singles.tile([P, KC], F32R)
    nc.sync.dma_start(vones_sb, vones)
    bq_sb = singles.tile([P, DC], F32)
    nc.sync.dma_start(bq_sb, bq)
    bk_sb = singles.tile([P, DC], F32)
    nc.sync.dma_start(bk_sb, bk)
    bo_sb = singles.tile([P, DC], F32)
    nc.sync.dma_start(bo_sb, bo)
    bf1_sb = singles.tile([P, FC], F32)
    nc.sync.dma_start(bf1_sb, bf1)
    bf2_sb = singles.tile([P, DC], F32)
    nc.sync.dma_start(bf2_sb, bf2)
    lcc_sb = singles.tile([P, KC], F32)
    nc.sync.dma_start(lcc_sb, lcck)
    bv_sb = singles.tile([P, D], F32)  # b_v broadcast to all partitions
    nc.sync.dma_start(bv_sb, bass.AP(tensor=bv.tensor, offset=0, ap=[[0, P], [1, D]]))
    eps_sb = singles.tile([1, 1], F32)
    nc.vector.memset(eps_sb, LN_EPS)

    def layer_norm_t(ctx_pool, ps_stat, ps_coef, src_tiles, dst, ncols, sq_pool,
                     src3=None, dst3=None, add_eng=None):
        """LayerNorm along feature dim for feature-major tiles.

        src_tiles: list of DC tiles/APs [128, ncols] (feature chunks)
        dst: [128, DC, ncols] output tile
        """
        sums = ps_stat.tile([1, ncols], F32, tag="stat")
        sumsq = ps_stat.tile([1, ncols], F32, tag="stat")
        for c in range(DC):
            xc = src_tiles[c]
            xsq = sq_pool.tile([P, ncols], F32R, tag="xsq")
            nc.scalar.square(xsq, xc)
            _mm(nc, sums, ones_col, xc, c == 0, c == DC - 1)
            _mm(nc, sumsq, ones_col, xsq, c == 0, c == DC - 1)
        # coeffs on one partition: rstd, shift = -mu*rstd
        mu = ctx_pool.tile([1, ncols], F32, tag="mu")
        nc.vector.tensor_scalar_mul(mu, sums, 1.0 / D)
        ex2 = ctx_pool.tile([1, ncols], F32, tag="ex2")
        nc.vector.tensor_scalar_mul(ex2, sumsq, 1.0 / D)
        var = ctx_pool.tile([1, ncols], F32, tag="var")
        nc.vector.tensor_mul(var, mu, mu)
        nc.vector.tensor_sub(var, ex2, var)
        sd = ctx_pool.tile([1, ncols], F32, tag="sd")
        nc.scalar.activation(sd, var, func=mybir.ActivationFunctionType.Sqrt,
                             bias=eps_sb, scale=1.0)
        rstd = ctx_pool.tile([1, ncols], F32R, tag="rstd")
        with nc.allow_low_precision(reason="f32r matmul operand"):
            nc.vector.reciprocal(rstd, sd)
        shift = ctx_pool.tile([1, ncols], F32R, tag="shift")
        nc.vector.tensor_mul(shift, mu, rstd)
        nc.vector.tensor_scalar_mul(shift, shift, -1.0)
        # broadcast to 128 partitions via K=1 matmul
        rstd_bc = ps_coef.tile([P, ncols], F32, tag="coef")
        shift_bc = ps_coef.tile([P, ncols], F32, tag="coef")
        _mm(nc, rstd_bc, ones_1x128, rstd, True, True)
        _mm(nc, shift_bc, ones_1x128, shift, True, True)
        if dst3 is not None:
            # one 3D op per pass; alternate the add between DVE and GpSimd so
            # neither engine serializes the block pipeline. GpSimd cannot read
            # PSUM, so stage the shift coefficients through SBUF for it.
            rb = rstd_bc.unsqueeze(1).to_broadcast(dst3.shape)
            if add_eng is nc.gpsimd:
                shift_sb = ctx_pool.tile([P, ncols], F32, tag="shift_sb",
                                         bufs=2)
                nc.scalar.copy(shift_sb, shift_bc)
                sb = shift_sb.unsqueeze(1).to_broadcast(dst3.shape)
            else:
                sb = shift_bc.unsqueeze(1).to_broadcast(dst3.shape)
            nc.vector.tensor_mul(dst3, src3, rb)
            add_eng.tensor_add(dst3, dst3, sb)
        else:
            for c in range(DC):
                nc.vector.tensor_mul(dst[:, c, :], src_tiles[c], rstd_bc)
                nc.vector.tensor_add(dst[:, c, :], dst[:, c, :], shift_bc)


    # persistent pools, allocated in reverse-release (stack) order
    vdram_pool = tc.alloc_tile_pool(name="vdram", bufs=1, space="DRAM")
    v_dram = vdram_pool.tile([KC, P, H, DH + 1], F32R)
    x2_pool = tc.alloc_tile_pool(name="x2p", bufs=1)
    x2acc = x2_pool.tile([P, DC, LQ], F32)
    x2 = x2_pool.tile([P, DC, LQ], F32R)
    kt_pool = tc.alloc_tile_pool(name="kt", bufs=1)
    k_t = kt_pool.tile([P, DC, L], F32R)  # [col-in-chunk, chunk, token]
    q_pool = tc.alloc_tile_pool(name="q", bufs=1)
    q_t = q_pool.tile([P, DC, LQ], F32R)
    normed_pool = tc.alloc_tile_pool(name="normed", bufs=1)
    normed_full = normed_pool.tile([P, DC, L], F32R)

    # =====================================================================
    # Phase A: LN1 over all tokens -> normed_full (feature-major, in place)
    # =====================================================================
    with (
        tc.tile_pool(name="ln1sq", bufs=2) as sq_pool,
        tc.tile_pool(name="ln1coef", bufs=1) as coef_small,
        tc.tile_pool(name="ps_stat", bufs=4, space="PSUM") as ps_stat,
        tc.tile_pool(name="ps_coef", bufs=2, space="PSUM") as ps_coef,
    ):
        for b in range(NBLK):
            blk = normed_full[:, :, b * BLK:(b + 1) * BLK]
            eng = nc.sync if b % 2 == 0 else nc.gpsimd
            eng.dma_start(blk, xt3[:, :, b * BLK:(b + 1) * BLK])
            layer_norm_t(coef_small, ps_stat, ps_coef,
                         [blk[:, c, :] for c in range(DC)], blk, BLK, sq_pool,
                         src3=blk, dst3=blk,
                         add_eng=nc.gpsimd if b % 2 == 0 else nc.vector)

    # =====================================================================
    # Phase C: own queries: LN1(own) -> q^T -> cosine-normalize * scaling
    # =====================================================================
    with (
        tc.tile_pool(name="qb", bufs=1) as qb_pool,
        tc.tile_pool(name="qsq", bufs=2) as qsq_pool,
        tc.tile_pool(name="qcoef", bufs=1) as qcoef,
        tc.tile_pool(name="wqstream", bufs=2) as wqstream,
    ):
        normed_own = qb_pool.tile([P, DC, LQ], F32R)
        nc.sync.dma_start(normed_own, xot3)
        with (
            tc.tile_pool(name="ps_stat2", bufs=2, space="PSUM") as ps_stat2,
            tc.tile_pool(name="ps_coef2", bufs=2, space="PSUM") as ps_coef2,
        ):
            layer_norm_t(qcoef, ps_stat2, ps_coef2,
                         [normed_own[:, c, :] for c in range(DC)], normed_own, LQ,
                         qsq_pool)
        with (
            tc.tile_pool(name="ps_mm2", bufs=2, space="PSUM") as ps_mm2,
            tc.tile_pool(name="ps_qn", bufs=2, space="PSUM") as ps_qn,
            tc.tile_pool(name="ps_qbc", bufs=2, space="PSUM") as ps_qbc,
        ):
            for m in range(DC):
                wqm = wqstream.tile([P, DC, P], F32R, tag="wq")
                nc.sync.dma_start(wqm, wq3[:, :, m * P:(m + 1) * P])
                ps = ps_mm2.tile([P, LQ], F32, tag="mm")
                for c in range(DC):
                    _mm(nc, ps, wqm[:, c, :], normed_own[:, c, :], c == 0,
                        c == DC - 1)
                nc.vector.tensor_scalar_add(q_t[:, m, :], ps, bq_sb[:, m:m + 1])
            # cosine-normalize q (x scaling folded into reciprocal)
            nsq = ps_qn.tile([H, LQ], F32, tag="qnsq")
            for m in range(DC):
                qsq = qsq_pool.tile([P, LQ], F32R, tag="xsq")
                nc.scalar.square(qsq, q_t[:, m, :])
                _mm(nc, nsq, selr_sb[:, m, :], qsq, m == 0, m == DC - 1)
            sd = qcoef.tile([H, LQ], F32, tag="qsd", bufs=2)
            nc.scalar.activation(sd, nsq,
                                 func=mybir.ActivationFunctionType.Sqrt,
                                 bias=0.0, scale=1.0)
            nc.vector.tensor_scalar_max(sd, sd, NORM_EPS)
            rec = qcoef.tile([H, LQ], F32R, tag="qrec", bufs=2)
            with nc.allow_low_precision(reason="f32r matmul operand"):
                nc.vector.reciprocal(rec, sd)
            nc.vector.tensor_scalar_mul(rec, rec, SCALING)
            for m in range(DC):
                bc = ps_qbc.tile([P, LQ], F32, tag="qbc")
                _mm(nc, bc, selb_sb[:, m, :], rec, True, True)
                nc.vector.tensor_mul(q_t[:, m, :], q_t[:, m, :], bc)

    # =====================================================================
    # Phase B: V (to DRAM scratch) then K^T + cosine-norm, block-pipelined
    # =====================================================================
    with (
        tc.tile_pool(name="wstream", bufs=2) as wstream,
        tc.tile_pool(name="vstage", bufs=3) as vstage,
        tc.tile_pool(name="knorm", bufs=2) as knorm_pool,
        tc.tile_pool(name="ps_mm", bufs=4, space="PSUM") as ps_mm,
        tc.tile_pool(name="ps_nrm", bufs=1, space="PSUM") as ps_nrm,
        tc.tile_pool(name="ps_nbc", bufs=1, space="PSUM") as ps_nbc,
    ):
        # V natural layout, block-major inside each quarter so the first
        # blocks of normed unblock V matmuls early
        QW = 256
        for n in range(4):
            wvn = wstream.tile([P, DC, QW], F32R, tag="wv")
            nc.gpsimd.dma_start(wvn, wv3[:, :, n * QW:(n + 1) * QW])
            for t in range(KC):
                ps = ps_mm.tile([P, QW], F32, tag="mmv", bufs=2)
                for c in range(DC):
                    _mm(nc, ps, normed_full[:, c, t * P:(t + 1) * P],
                        wvn[:, c, :], c == 0, c == DC - 1)
                stag = vstage.tile([P, 4, DH], F32R, tag="vstage")
                nc.vector.tensor_add(
                    stag, ps.rearrange("p (h d) -> p h d", d=DH),
                    bv_sb[:, n * QW:(n + 1) * QW].rearrange("p (h d) -> p h d",
                                                            d=DH))
                nc.gpsimd.dma_start(v_dram[t, :, n * 4:(n + 1) * 4, 0:DH], stag)
        # K block-outer with inline cosine-normalization, so attention's
        # exp work unblocks per block instead of all at the end
        for b in range(NBLK):
            for m in range(DC):
                wkm = wstream.tile([P, DC, P], F32R, tag="wk")
                nc.sync.dma_start(wkm, wk3[:, :, m * P:(m + 1) * P])
                ps = ps_mm.tile([P, BLK], F32, tag="mm")
                for c in range(DC):
                    _mm(nc, ps, wkm[:, c, :],
                        normed_full[:, c, b * BLK:(b + 1) * BLK], c == 0,
                        c == DC - 1)
                nc.vector.tensor_scalar_add(k_t[:, m, b * BLK:(b + 1) * BLK],
                                            ps, bk_sb[:, m:m + 1])
            nsq = ps_nrm.tile([H, BLK], F32, tag="nsq")
            for m in range(DC):
                ksq = knorm_pool.tile([P, BLK], F32R, tag="ksq")
                nc.scalar.square(ksq, k_t[:, m, b * BLK:(b + 1) * BLK])
                _mm(nc, nsq, selr_sb[:, m, :], ksq, m == 0, m == DC - 1)
            sd = knorm_pool.tile([H, BLK], F32, tag="ksd")
            nc.scalar.activation(sd, nsq,
                                 func=mybir.ActivationFunctionType.Sqrt,
                                 bias=0.0, scale=1.0)
            nc.vector.tensor_scalar_max(sd, sd, NORM_EPS)
            rec = knorm_pool.tile([H, BLK], F32R, tag="krec")
            with nc.allow_low_precision(reason="f32r matmul operand"):
                nc.vector.reciprocal(rec, sd)
            for m in range(DC):
                bc = ps_nbc.tile([P, BLK], F32, tag="nbc")
                _mm(nc, bc, selb_sb[:, m, :], rec, True, True)
                nc.vector.tensor_mul(k_t[:, m, b * BLK:(b + 1) * BLK],
                                     k_t[:, m, b * BLK:(b + 1) * BLK], bc)

    normed_pool.release()

    # =====================================================================
    # Phase D: attention per head-pair, with the out-projection folded in
    # (partial products accumulated into x2acc via DVE)
    # =====================================================================
    with (
        tc.tile_pool(name="exp", bufs=2) as exp_pool,
        tc.tile_pool(name="vsb", bufs=2) as vsb_pool,
        tc.tile_pool(name="rsc", bufs=2) as rsc_pool,
        tc.tile_pool(name="apair", bufs=2) as apair_pool,
        tc.tile_pool(name="wostream", bufs=2) as wostream,
        tc.tile_pool(name="ps_sc", bufs=2, space="PSUM") as ps_sc,
        tc.tile_pool(name="ps_acc", bufs=1, space="PSUM") as ps_acc,
        tc.tile_pool(name="ps_rbc", bufs=1, space="PSUM") as ps_rbc,
        tc.tile_pool(name="ps_op", bufs=2, space="PSUM") as ps_op,
    ):
        for m in range(DC):
            vp = vsb_pool.tile([P, KC, 2, DH + 1], F32R, tag="vp")
            for j in range(2):
                nc.gpsimd.dma_start(
                    vp[:, :, j, 0:DH],
                    v_dram[:, :, 2 * m + j, 0:DH].rearrange("k p d -> p k d"))
                nc.gpsimd.dma_start(
                    vp[:, :, j, DH:DH + 1],
                    vones_sb.rearrange("p (h o) -> p h o", o=1))
            eh = exp_pool.tile([P, KC, 2 * LQ], F32R, tag="exp")
            for kc in range(KC):
                # each head's scores go to a separate PSUM bank: fp32r matmul
                # writes at mid-bank free offsets fault on hardware
                ps = ps_sc.tile([P, 2, 2 * LQ], F32, tag="sc")
                for j in range(2):
                    _mm(nc, ps[:, j, 0:LQ],
                        k_t[j * DH:(j + 1) * DH, m, kc * P:(kc + 1) * P],
                        q_t[j * DH:(j + 1) * DH, m, :], True, True)
                nc.scalar.activation(
                    eh[:, kc, :].rearrange("p (j q) -> p j q", j=2),
                    ps[:, :, 0:LQ],
                    func=mybir.ActivationFunctionType.Exp,
                    bias=lcc_sb[:, kc:kc + 1], scale=1.0)
            attn_pair = apair_pool.tile([P, LQ], F32R, tag="apair")
            for j in range(2):
                acc = ps_acc.tile([DH + 1, LQ], F32, tag="acc")
                for kc in range(KC):
                    _mm(nc, acc, vp[:, kc, j, :],
                        eh[:, kc, j * LQ:(j + 1) * LQ], kc == 0, kc == KC - 1)
                recip = rsc_pool.tile([1, LQ], F32R, tag="recip")
                with nc.allow_low_precision(reason="f32r matmul operand"):
                    nc.vector.reciprocal(recip, acc[DH:DH + 1, :])
                rbc = ps_rbc.tile([DH, LQ], F32, tag="rbc")
                _mm(nc, rbc, ones_1x128[:, 0:DH], recip, True, True)
                rbc_sb = rsc_pool.tile([DH, LQ], F32, tag="rbcsb")
                nc.vector.tensor_copy(rbc_sb, rbc)
                nc.vector.tensor_mul(attn_pair[j * DH:(j + 1) * DH, :],
                                     acc[0:DH, :], rbc_sb)
            # out-projection partial for this pair-chunk of attn
            wom = wostream.tile([P, DC, P], F32R, tag="wo")
            nc.sync.dma_start(wom, wo3.rearrange("p c n -> p c n")[
                :, m, :].rearrange("p (o n) -> p o n", n=P))
            for o in range(DC):
                pso = ps_op.tile([P, LQ], F32, tag="op")
                _mm(nc, pso, wom[:, o, :], attn_pair, True, True)
                if m == 0:
                    nc.vector.tensor_copy(x2acc[:, o, :], pso)
                else:
                    nc.vector.tensor_add(x2acc[:, o, :], x2acc[:, o, :], pso)

    q_pool.release()
    kt_pool.release()

    # =====================================================================
    # Phase E: residual -> x2; LN2; FFN (ff2 single-pass, half-packed psum)
    # =====================================================================
    with (
        tc.tile_pool(name="xo2p", bufs=1) as xo2_pool,
        tc.tile_pool(name="ffsq", bufs=2) as ffsq_pool,
        tc.tile_pool(name="ffcoef", bufs=2) as ffcoef,
        tc.tile_pool(name="ht", bufs=1) as ht_pool,
        tc.tile_pool(name="wf1s", bufs=3) as wf1s,
        tc.tile_pool(name="wf2s", bufs=3) as wf2s,
        tc.tile_pool(name="outsb", bufs=2) as outsb_pool,
    ):
        xo2 = xo2_pool.tile([P, DC, LQ], F32R)
        nc.sync.dma_start(xo2, xot3)
        for o in range(DC):
            nc.vector.tensor_scalar_add(x2[:, o, :], x2acc[:, o, :],
                                        bo_sb[:, o:o + 1])
            nc.vector.tensor_add(x2[:, o, :], x2[:, o, :], xo2[:, o, :])
        normed2 = xo2_pool.tile([P, DC, LQ], F32R)
        with (
            tc.tile_pool(name="ps_stat3", bufs=2, space="PSUM") as ps_stat3,
            tc.tile_pool(name="ps_coef3", bufs=2, space="PSUM") as ps_coef3,
        ):
            layer_norm_t(ffcoef, ps_stat3, ps_coef3,
                         [x2[:, c, :] for c in range(DC)], normed2, LQ,
                         ffsq_pool)
        ps_mm3 = tc.alloc_tile_pool(name="ps_mm3", bufs=3, space="PSUM")
        ps_ff2 = tc.alloc_tile_pool(name="ps_ff2", bufs=4, space="PSUM")
        h_t = ht_pool.tile([P, FC, LQ], F32R)
        wf24 = wf23.rearrange("p c (g n) -> p c g n", g=2)  # [128,32,2,512]
        for f in range(FC):
            wf1m = wf1s.tile([P, DC, P], F32R, tag="wf1")
            weng = nc.sync if f % 2 == 0 else nc.gpsimd
            weng.dma_start(wf1m, wf13[:, :, f * P:(f + 1) * P])
            ps = ps_mm3.tile([P, LQ], F32, tag="mm")
            for c in range(DC):
                _mm(nc, ps, wf1m[:, c, :], normed2[:, c, :], c == 0, c == DC - 1)
            nc.scalar.activation(h_t[:, f, :], ps, func=GELU_FUNC,
                                 bias=bf1_sb[:, f:f + 1], scale=1.0)
        # ff2: f-outer accumulation in two 4-output passes; pass 1 pipelines
        # with ff1 chunk by chunk
        for g in range(2):
            accs = [ps_ff2.tile([P, LQ], F32, tag="ff2acc",
                                name=f"ff2acc_{g}_{i}") for i in range(4)]
            for f in range(FC):
                wf2m = wf2s.tile([P, 4, P], F32R, tag="wf2")
                weng2 = nc.gpsimd if f % 2 == 0 else nc.sync
                weng2.dma_start(wf2m, wf24[:, f, g, :].rearrange(
                    "p (i n) -> p i n", n=P))
                for i in range(4):
                    _mm(nc, accs[i], wf2m[:, i, :], h_t[:, f, :],
                        f == 0, f == FC - 1)
            for i in range(4):
                mcol = g * 4 + i
                osb = outsb_pool.tile([P, LQ], F32, tag="osb")
                nc.vector.tensor_scalar_add(osb, accs[i], bf2_sb[:, mcol:mcol + 1])
                nc.vector.tensor_add(osb, osb, x2[:, mcol, :])
                nc.sync.dma_start(out3[:, mcol, :], osb)
        ps_ff2.release()
        ps_mm3.release()

    x2_pool.release()
    vdram_pool.release()
    singles.release()


_CACHED = None


def build():
    global _CACHED
    if _CACHED is None:
        nc = bacc.Bacc("TRN2", target_bir_lowering=False, debug=False)
        with tile.TileContext(nc) as tc:
            emit(tc)
        nc.compile()
        _CACHED = nc
    return _CACHED


def _onesc_matrix():
    o = np.zeros((P, 3), np.float32)
    o[:, 0] = 1.0
    o[0:DH, 1] = 1.0
    o[DH:P, 2] = 1.0
    return o


def _selr_matrix():
    # [P, DC*H]: selr[p, m*16+h] = 1 iff h == 2m + (p >= 64)
    s = np.zeros((P, DC, H), np.float32)
    for m in range(DC):
        s[0:DH, m, 2 * m] = 1.0
        s[DH:P, m, 2 * m + 1] = 1.0
    return np.ascontiguousarray(s.reshape(P, P))


def _selb_matrix():
    # [H, DC*P]: selb[h, m*128+p] = 1 iff h == 2m + (p >= 64)
    s = np.zeros((H, DC, P), np.float32)
    for m in range(DC):
        s[2 * m, m, 0:DH] = 1.0
        s[2 * m + 1, m, DH:P] = 1.0
    return np.ascontiguousarray(s.reshape(H, DC * P))


def prep_inputs(inputs):
    """Host-side preprocessing: transpose x, split/fold weights, bias layouts."""
    f = np.float32
    x = np.asarray(inputs["x"], f)
    lcc = np.asarray(inputs["lcc_values"], f)
    w_qkv = np.asarray(inputs["w_qkv"], f)
    b_qkv = np.asarray(inputs["b_qkv"], f)
    ln1_g = np.asarray(inputs["ln1_g"], f)
    ln1_b = np.asarray(inputs["ln1_b"], f)
    ln2_g = np.asarray(inputs["ln2_g"], f)
    ln2_b = np.asarray(inputs["ln2_b"], f)
    w_ff1 = np.asarray(inputs["w_ff1"], f)
    b_ff1 = np.asarray(inputs["b_ff1"], f)

    def chunked(b):  # [D] -> [128, DC] with chunk c in column c
        return np.ascontiguousarray(b.reshape(-1, P).T)

    xt = np.ascontiguousarray(x.T)
    shared = {
        "xt": xt,
        "wq": np.ascontiguousarray(ln1_g[:, None] * w_qkv[:, 0:D]),
        "wk": np.ascontiguousarray(ln1_g[:, None] * w_qkv[:, D:2 * D]),
        "wv": np.ascontiguousarray(ln1_g[:, None] * w_qkv[:, 2 * D:3 * D]),
        "wo": np.ascontiguousarray(np.asarray(inputs["w_out"], f)),
        "wf1": np.ascontiguousarray(ln2_g[:, None] * w_ff1),
        "wf2": np.ascontiguousarray(np.asarray(inputs["w_ff2"], f)),
        "bq": chunked(b_qkv[0:D] + ln1_b @ w_qkv[:, 0:D]),
        "bk": chunked(b_qkv[D:2 * D] + ln1_b @ w_qkv[:, D:2 * D]),
        "bv": np.ascontiguousarray(b_qkv[2 * D:3 * D] + ln1_b @ w_qkv[:, 2 * D:3 * D]),
        "bo": chunked(np.asarray(inputs["b_out"], f)),
        "bf1": chunked(b_ff1 + ln2_b @ w_ff1),
        "bf2": chunked(np.asarray(inputs["b_ff2"], f)),
        "lcck": np.ascontiguousarray((lcc * (0.5 * LCC)).reshape(KC, P).T),
        "selr": _selr_matrix(),
        "selb": _selb_matrix(),
        "onesc": _onesc_matrix(),
        "ones1r": np.ones((1, P), np.float32),
        "vones": np.ones((P, KC), np.float32),
    }
    in_maps = []
    for c in range(NCORES):
        m = dict(shared)
        m["xot"] = np.ascontiguousarray(xt[:, c * LQ:(c + 1) * LQ])
        in_maps.append(m)
    return in_maps


def kernel(**inputs):
    nc = build()
    in_maps = prep_inputs(inputs)
    res = run_bass_kernel_spmd(nc, in_maps, core_ids=list(range(NCORES)))
    out = np.concatenate([res.results[c]["out_t"] for c in range(NCORES)], axis=1)
    return np.ascontiguousarray(out.T).astype(np.float32)



# revision 2
# speedup vs baseline: 1.1382x; 1.1382x over previous
"""Trainium2 Bass kernel for EnhancedMultiHeadSelfAttention.

Sharding: tensor-parallel attention by heads (core c owns heads 2c, 2c+1 over
ALL 2048 tokens) + sequence-parallel FFN (core c owns tokens 256c..256c+255).
The out-projection partial [2048, 1024] is combined with a single DRAM
ReduceScatter (layout [8, 1024, 256] so the flat-chunk scatter hands each core
its own 256 token columns, feature-major).

Math notes (beyond the baseline's):
 - LN1 is folded into the QKV matmuls: with per-token mean mu and std sd,
   W^T LN(x) = (W diag(g))^T x * r - mu*r*cg + cb  (r = 1/sd, cg = W^T g,
   cb = b + W^T ln1_b).  Dividing by r>0 is free for Q and K (cosine attention
   normalizes them per token), so q' = Wg^T x + cg*(-mu) + cb*sd — one K=2
   rank-1 matmul accumulated into the projection PSUM group. V keeps the same
   rank-1 term and a final per-token r multiply (applied post-transpose where
   tokens sit on partitions).
 - The per-token r for V reaches token-partitions via a tiny K=1 transposing
   matmul (lhsT = r row-slice, rhs = [1,1] ones).
 - softmax needs no max-subtraction; only the key-side lcc bias matters; it is
   applied MULTIPLICATIVELY (exp(s+b) = exp(s)*exp(b)) by scaling V's rows and
   the appended denominator column by host-computed exp(b_k), so the exp
   activation needs no bias and can batch two key-chunks per instruction.
 - FFN weights, h, normed2, and the ReduceScatter payload are bf16 (PSUM
   accumulation stays fp32).
"""

import ml_dtypes
import numpy as np

import concourse.bass as bass
import concourse.tile as tile
from concourse import bacc, mybir
from concourse.bass_utils import run_bass_kernel_spmd

F32 = mybir.dt.float32
F32R = mybir.dt.float32r
BF16 = mybir.dt.bfloat16

L = 2048          # sequence length
D = 1024          # model dim
H = 16            # heads (2 per core)
DH = 64           # head dim
FF = 4096         # ffn hidden
P = 128           # partitions
NCORES = 8
LQ = L // NCORES  # 256 own tokens per core (FFN + output)
DC = D // P       # 8 d-model chunks
FC = FF // P      # 32 ffn chunks
KC = L // P       # 16 key chunks of 128
NBLK = 4          # token blocks of 512
BLK = L // NBLK   # 512

GELU_FUNC = mybir.ActivationFunctionType.Gelu

LN_EPS = 1e-5
NORM_EPS = 1e-12
SCALING = DH ** -0.5
LCC = 0.1


def _mm(nc, out, lhsT, rhs, start, stop):
    assert lhsT.dtype == rhs.dtype and lhsT.dtype in (F32R, BF16), \
        (lhsT.dtype, rhs.dtype)
    nc.tensor.matmul(out, lhsT, rhs, start=start, stop=stop)


def emit(tc):
    nc = tc.nc

    xt = nc.dram_tensor("xt", [D, L], F32R, kind="ExternalInput").ap()
    xot = nc.dram_tensor("xot", [D, LQ], F32R, kind="ExternalInput").ap()
    wq2 = nc.dram_tensor("wq2", [D, P], F32R, kind="ExternalInput").ap()
    wk2 = nc.dram_tensor("wk2", [D, P], F32R, kind="ExternalInput").ap()
    wv2 = nc.dram_tensor("wv2", [D, P], F32R, kind="ExternalInput").ap()
    wo2 = nc.dram_tensor("wo2", [P, D], F32R, kind="ExternalInput").ap()
    wf1 = nc.dram_tensor("wf1", [D, FF], BF16, kind="ExternalInput").ap()
    wf2 = nc.dram_tensor("wf2", [FF, D], BF16, kind="ExternalInput").ap()
    cgbq = nc.dram_tensor("cgbq", [2, P], F32R, kind="ExternalInput").ap()
    cgbk = nc.dram_tensor("cgbk", [2, P], F32R, kind="ExternalInput").ap()
    cgbv = nc.dram_tensor("cgbv", [2, P], F32R, kind="ExternalInput").ap()
    bo = nc.dram_tensor("bo", [P, DC], F32, kind="ExternalInput").ap()
    bf1 = nc.dram_tensor("bf1", [P, FC], F32, kind="ExternalInput").ap()
    bf2 = nc.dram_tensor("bf2", [P, DC], F32, kind="ExternalInput").ap()
    explcc = nc.dram_tensor("explcc", [P, KC], F32, kind="ExternalInput").ap()
    sel2 = nc.dram_tensor("sel2", [P, 2], F32R, kind="ExternalInput").ap()
    selb2 = nc.dram_tensor("selb2", [2, P], F32R, kind="ExternalInput").ap()
    ident = nc.dram_tensor("ident", [P, P], F32R, kind="ExternalInput").ap()
    ones1r = nc.dram_tensor("ones1r", [1, P], F32R, kind="ExternalInput").ap()
    ones1f = nc.dram_tensor("ones1f", [1, P], F32, kind="ExternalInput").ap()
    onesc = nc.dram_tensor("onesc", [P, 1], F32R, kind="ExternalInput").ap()
    out_t = nc.dram_tensor("out_t", [D, LQ], F32, kind="ExternalOutput").ap()

    xt3 = xt.rearrange("(c p) t -> p c t", p=P)        # [128, 8, 2048]
    xot3 = xot.rearrange("(c p) t -> p c t", p=P)      # [128, 8, 256]
    wq3 = wq2.rearrange("(c p) n -> p c n", p=P)       # [128, 8, 128]
    wk3 = wk2.rearrange("(c p) n -> p c n", p=P)
    wv3 = wv2.rearrange("(c p) n -> p c n", p=P)
    wf13 = wf1.rearrange("(c p) n -> p c n", p=P)      # [128, 8, 4096]
    wf23 = wf2.rearrange("(c p) n -> p c n", p=P)      # [128, 32, 1024]
    out3 = out_t.rearrange("(c p) t -> p c t", p=P)    # [128, 8, 256]

    # ---- persistent small constants -------------------------------------
    singles = tc.alloc_tile_pool(name="singles", bufs=1)
    ones_1x128 = singles.tile([1, P], F32R)
    nc.sync.dma_start(ones_1x128, ones1r)
    ones1f_sb = singles.tile([1, P], F32)
    nc.sync.dma_start(ones1f_sb, ones1f)
    ones_col = singles.tile([P, 1], F32R)
    nc.sync.dma_start(ones_col, onesc)
    sel2_sb = singles.tile([P, 2], F32R)
    nc.sync.dma_start(sel2_sb, sel2)
    selb2_sb = singles.tile([2, P], F32R)
    nc.sync.dma_start(selb2_sb, selb2)
    ident_sb = singles.tile([P, P], F32R)
    nc.sync.dma_start(ident_sb, ident)
    cgbq_sb = singles.tile([2, P], F32R)
    nc.sync.dma_start(cgbq_sb, cgbq)
    cgbk_sb = singles.tile([2, P], F32R)
    nc.sync.dma_start(cgbk_sb, cgbk)
    cgbv_sb = singles.tile([2, P], F32R)
    nc.sync.dma_start(cgbv_sb, cgbv)
    bo_sb = singles.tile([P, DC], F32)
    nc.sync.dma_start(bo_sb, bo)
    bf1_sb = singles.tile([P, FC], F32)
    nc.sync.dma_start(bf1_sb, bf1)
    bf2_sb = singles.tile([P, DC], F32)
    nc.sync.dma_start(bf2_sb, bf2)
    explcc_sb = singles.tile([P, KC], F32)
    nc.sync.dma_start(explcc_sb, explcc)
    wo2_sb = singles.tile([P, DC, P], F32R)
    nc.sync.dma_start(wo2_sb, wo2.rearrange("p (c n) -> p c n", n=P))
    wq_sb = singles.tile([P, DC, P], F32R)
    nc.sync.dma_start(wq_sb, wq3)
    wk_sb = singles.tile([P, DC, P], F32R)
    nc.sync.dma_start(wk_sb, wk3)
    wv_sb = singles.tile([P, DC, P], F32R)
    nc.sync.dma_start(wv_sb, wv3)
    eps_sb = singles.tile([1, 1], F32)
    nc.vector.memset(eps_sb, LN_EPS)

    # persistent activation tiles (released before FFN where possible)
    qk_pool = tc.alloc_tile_pool(name="qk", bufs=1)
    q_t = qk_pool.tile([P, L], F32R)     # [2 heads x 64 dims, tokens]
    k_t = qk_pool.tile([P, L], F32R)
    v_tm = qk_pool.tile([P, KC, 2 * (DH + 1)], F32R)  # [keys, kc, (d+1)*2h]

    # DRAM scratch: out-proj partial, laid out for flat-chunk ReduceScatter
    podram_pool = tc.alloc_tile_pool(name="podram", bufs=1, space="DRAM")
    po_dram = podram_pool.tile([NCORES, D, LQ], BF16)
    rs_pool = tc.alloc_tile_pool(name="rsdram", bufs=1, space="DRAM")
    rs_dram = rs_pool.tile([D, LQ], BF16)

    # =====================================================================
    # Phase 1: stats + QKV projections for the core's 2 heads, all tokens
    # =====================================================================
    with (
        tc.tile_pool(name="xb", bufs=2) as xb_pool,
        tc.tile_pool(name="sq", bufs=2) as sq_pool,
        tc.tile_pool(name="smalls", bufs=2) as smalls,
        tc.tile_pool(name="vstage", bufs=2) as vstage_pool,
        tc.tile_pool(name="ps_stat", bufs=1, space="PSUM") as ps_stat,
        tc.tile_pool(name="ps_mm", bufs=3, space="PSUM") as ps_mm,
        tc.tile_pool(name="ps_nrm", bufs=1, space="PSUM") as ps_nrm,
        tc.tile_pool(name="ps_vt", bufs=1, space="PSUM") as ps_vt_pool,
    ):
        for b in range(NBLK):
            sl = slice(b * BLK, (b + 1) * BLK)
            xb = xb_pool.tile([P, DC, BLK], F32R, tag="xb")
            nc.sync.dma_start(xb, xt3[:, :, sl])
            # token stats: sums (row 0) and sum-of-squares (row 1) via
            # ones-column matmuls into disjoint partition rows of one bank
            sums = ps_stat.tile([1, BLK], F32, tag="sums")
            sumsq = ps_stat.tile([1, BLK], F32, tag="sumsq")
            xsq = sq_pool.tile([P, DC, BLK], F32R, tag="xsq")
            nc.scalar.square(xsq, xb)
            for c in range(DC):
                _mm(nc, sums, ones_col, xb[:, c, :], c == 0, c == DC - 1)
                _mm(nc, sumsq, ones_col, xsq[:, c, :], c == 0, c == DC - 1)
            # rhs2 = [-mu; sd] for the rank-1 LN fold; r = 1/sd for V
            mu = smalls.tile([1, BLK], F32, tag="mu")
            nc.vector.tensor_scalar_mul(mu, sums, 1.0 / D)
            ex2 = smalls.tile([1, BLK], F32, tag="ex2")
            nc.vector.tensor_scalar_mul(ex2, sumsq, 1.0 / D)
            var = smalls.tile([1, BLK], F32, tag="var")
            nc.vector.tensor_mul(var, mu, mu)
            nc.vector.tensor_sub(var, ex2, var)
            rhs2 = smalls.tile([2, BLK], F32R, tag="rhs2")
            sd0 = smalls.tile([1, BLK], F32, tag="sd0")
            nc.scalar.activation(sd0, var,
                                 func=mybir.ActivationFunctionType.Sqrt,
                                 bias=eps_sb, scale=1.0)
            with nc.allow_low_precision(reason="f32r matmul operand"):
                nc.vector.tensor_scalar_mul(rhs2[0:1, :], mu, -1.0)
            nc.gpsimd.dma_start(rhs2[1:2, :], sd0)
            r_row = smalls.tile([1, BLK], F32, tag="rrow")
            nc.vector.reciprocal(r_row, sd0)

            # Q / K with cosine normalization folded
            for (wsb, cgb, dst, scaled) in (
                (wq_sb, cgbq_sb, q_t, True),
                (wk_sb, cgbk_sb, k_t, False),
            ):
                ps = ps_mm.tile([P, BLK], F32, tag="mm")
                for c in range(DC):
                    _mm(nc, ps, wsb[:, c, :], xb[:, c, :], c == 0, False)
                _mm(nc, ps, cgb, rhs2, False, True)
                psq = sq_pool.tile([P, BLK], F32R, tag="psq")
                nc.scalar.square(psq, ps)
                nsq = ps_nrm.tile([2, BLK], F32, tag="aux")
                _mm(nc, nsq, sel2_sb, psq, True, True)
                sdq = smalls.tile([2, BLK], F32, tag="sdq")
                nc.scalar.activation(sdq, nsq,
                                     func=mybir.ActivationFunctionType.Sqrt,
                                     bias=0.0, scale=1.0)
                nc.vector.tensor_scalar_max(sdq, sdq, NORM_EPS)
                rec = smalls.tile([2, BLK], F32R, tag="rec")
                with nc.allow_low_precision(reason="f32r matmul operand"):
                    nc.vector.reciprocal(rec, sdq)
                if scaled:
                    nc.vector.tensor_scalar_mul(rec, rec, SCALING)
                rbc = ps_nrm.tile([P, BLK], F32, tag="rbc")
                _mm(nc, rbc, selb2_sb, rec, True, True)
                rbc_sb = smalls.tile([P, BLK], F32, tag="rbcsb")
                nc.vector.tensor_copy(rbc_sb, rbc)
                nc.vector.tensor_mul(dst[:, sl], ps, rbc_sb)

            # V: rank-1 fold, PE transpose to token-major, then r multiply
            ps = ps_mm.tile([P, BLK], F32, tag="mm")
            for c in range(DC):
                _mm(nc, ps, wv_sb[:, c, :], xb[:, c, :], c == 0, False)
            _mm(nc, ps, cgbv_sb, rhs2, False, True)
            vstage = vstage_pool.tile([P, BLK], F32R, tag="vstage")
            nc.vector.tensor_copy(vstage, ps)
            for t in range(4):
                kc = b * 4 + t
                tsl = slice(t * P, (t + 1) * P)
                vt = ps_vt_pool.tile([P, BLK], F32R, tag="vt")
                nc.tensor.matmul(vt[:, 0:P], vstage[:, tsl], ident_sb,
                                 is_transpose=True, start=True, stop=True)
                rtm = ps_nrm.tile([P, 1], F32, tag="aux")
                nc.tensor.matmul(rtm, r_row[:, tsl], ones1f_sb[:, 0:1],
                                 start=True, stop=True)
                # fold exp(key-side lcc bias) into V rows and the ones column
                rtme = smalls.tile([P, 1], F32, tag="rtme")
                nc.vector.tensor_mul(rtme, explcc_sb[:, kc:kc + 1], rtm)
                nc.vector.tensor_scalar_mul(v_tm[:, kc, 0:DH], vt[:, 0:DH],
                                            rtme)
                nc.vector.tensor_scalar_mul(v_tm[:, kc, DH + 1:2 * DH + 1],
                                            vt[:, DH:2 * DH], rtme)
            nc.gpsimd.tensor_copy(
                v_tm[:, b * 4:(b + 1) * 4, DH:DH + 1],
                explcc_sb[:, b * 4:(b + 1) * 4].unsqueeze(2))
            nc.gpsimd.tensor_copy(
                v_tm[:, b * 4:(b + 1) * 4, 2 * DH + 1:],
                explcc_sb[:, b * 4:(b + 1) * 4].unsqueeze(2))

    # =====================================================================
    # Phase 2: attention for 2 heads over all queries + out-proj partials
    # (first half of wf1 prefetches in the background; rest streams in ff1)
    # =====================================================================
    wf1_pool = tc.alloc_tile_pool(name="wf1sb", bufs=1)
    wf1_sb = wf1_pool.tile([P, DC, FF], BF16)
    for sl4 in range(4):
        nc.sync.dma_start(wf1_sb[:, :, sl4 * FF // 4:(sl4 + 1) * FF // 4],
                          wf13[:, :, sl4 * FF // 4:(sl4 + 1) * FF // 4])

    with (
        tc.tile_pool(name="eh", bufs=3) as eh_pool,
        tc.tile_pool(name="apair", bufs=2) as apair_pool,
        tc.tile_pool(name="rsc", bufs=2) as rsc_pool,
        tc.tile_pool(name="postage", bufs=3) as postage_pool,
        tc.tile_pool(name="ps_sc", bufs=2, space="PSUM") as ps_sc,
        tc.tile_pool(name="ps_acc", bufs=2, space="PSUM") as ps_acc,
        tc.tile_pool(name="ps_rbc", bufs=1, space="PSUM") as ps_rbc,
        tc.tile_pool(name="ps_op", bufs=1, space="PSUM") as ps_op,
    ):
        for qb in range(NBLK):
            qsl = slice(qb * BLK, (qb + 1) * BLK)
            apair = apair_pool.tile([P, BLK], F32R, tag="apair")
            for j in range(2):
                acc = ps_acc.tile([DH + 1, BLK], F32, tag="acc")
                for kc2 in range(KC // 2):
                    ps = ps_sc.tile([P, 2, BLK], F32, tag="sc")
                    eh = eh_pool.tile([P, 2, BLK], F32R, tag="eh")
                    for u in range(2):
                        kc = 2 * kc2 + u
                        _mm(nc, ps[:, u, :],
                            k_t[j * DH:(j + 1) * DH, kc * P:(kc + 1) * P],
                            q_t[j * DH:(j + 1) * DH, qsl], True, True)
                    nc.scalar.activation(eh, ps,
                                         func=mybir.ActivationFunctionType.Exp,
                                         bias=0.0, scale=1.0)
                    for u in range(2):
                        kc = 2 * kc2 + u
                        _mm(nc, acc,
                            v_tm[:, kc, j * (DH + 1):(j + 1) * (DH + 1)],
                            eh[:, u, :], kc == 0, kc == KC - 1)
                recip = rsc_pool.tile([1, BLK], F32R, tag="recip")
                with nc.allow_low_precision(reason="f32r matmul operand"):
                    nc.vector.reciprocal(recip, acc[DH:DH + 1, :])
                rbc = ps_rbc.tile([DH, BLK], F32, tag="rbc")
                _mm(nc, rbc, ones_1x128[:, 0:DH], recip, True, True)
                rbc_sb = rsc_pool.tile([DH, BLK], F32, tag="rbcsb")
                nc.vector.tensor_copy(rbc_sb, rbc)
                nc.vector.tensor_mul(apair[j * DH:(j + 1) * DH, :],
                                     acc[0:DH, :], rbc_sb)
            # out-projection partial for this query block
            postage = postage_pool.tile([P, DC, BLK], BF16, tag="po")
            for o in range(DC):
                pso = ps_op.tile([P, BLK], F32, tag="op")
                _mm(nc, pso, wo2_sb[:, o, :], apair, True, True)
                nc.vector.tensor_copy(postage[:, o, :], pso)
            for o in range(DC):
                nc.sync.dma_start(
                    po_dram[2 * qb:2 * qb + 2, o * P:(o + 1) * P, :]
                    .rearrange("s p t -> p s t"),
                    postage[:, o, :].rearrange("p (s t) -> p s t", t=LQ))

    # =====================================================================
    # Phase 3: ReduceScatter -> own 256 token columns, feature-major
    # =====================================================================
    nc.gpsimd.collective_compute(
        "ReduceScatter",
        mybir.AluOpType.add,
        replica_groups=[list(range(NCORES))],
        ins=[po_dram],
        outs=[rs_dram],
    )

    # =====================================================================
    # Phase 4: residual; LN2; FFN (seq-parallel, wf1 resident, wf2 streamed)
    # =====================================================================
    with (
        tc.tile_pool(name="x2p", bufs=1) as x2_pool,
        tc.tile_pool(name="ffsq", bufs=2) as ffsq_pool,
        tc.tile_pool(name="ffsm", bufs=2) as ffsm,
        tc.tile_pool(name="ht", bufs=1) as ht_pool,
        tc.tile_pool(name="wf2s", bufs=3) as wf2s,
        tc.tile_pool(name="outsb", bufs=2) as outsb_pool,
    ):
        x2 = x2_pool.tile([P, DC, LQ], F32R)
        xo2 = x2_pool.tile([P, DC, LQ], F32R)
        nc.sync.dma_start(xo2, xot3)
        rs_sb = x2_pool.tile([P, DC, LQ], BF16)
        nc.sync.dma_start(rs_sb, rs_dram.rearrange("(c p) t -> p c t", p=P))
        for o in range(DC):
            nc.vector.tensor_scalar_add(x2[:, o, :], rs_sb[:, o, :],
                                        bo_sb[:, o:o + 1])
            nc.vector.tensor_add(x2[:, o, :], x2[:, o, :], xo2[:, o, :])
        # LN2 (plain: stats -> coefs -> broadcast -> apply)
        normed2 = x2_pool.tile([P, DC, LQ], BF16)
        with (
            tc.tile_pool(name="ps_stat3", bufs=1, space="PSUM") as ps_stat3,
            tc.tile_pool(name="ps_coef3", bufs=2, space="PSUM") as ps_coef3,
        ):
            sums = ps_stat3.tile([1, LQ], F32, tag="sums")
            sumsq = ps_stat3.tile([1, LQ], F32, tag="sumsq")
            for c in range(DC):
                xsq = ffsq_pool.tile([P, LQ], F32R, tag="xsq")
                nc.scalar.square(xsq, x2[:, c, :])
                _mm(nc, sums, ones_col, x2[:, c, :], c == 0, c == DC - 1)
                _mm(nc, sumsq, ones_col, xsq, c == 0, c == DC - 1)
            mu = ffsm.tile([1, LQ], F32, tag="mu")
            nc.vector.tensor_scalar_mul(mu, sums, 1.0 / D)
            ex2 = ffsm.tile([1, LQ], F32, tag="ex2")
            nc.vector.tensor_scalar_mul(ex2, sumsq, 1.0 / D)
            var = ffsm.tile([1, LQ], F32, tag="var")
            nc.vector.tensor_mul(var, mu, mu)
            nc.vector.tensor_sub(var, ex2, var)
            sd = ffsm.tile([1, LQ], F32, tag="sd")
            nc.scalar.activation(sd, var,
                                 func=mybir.ActivationFunctionType.Sqrt,
                                 bias=eps_sb, scale=1.0)
            rstd = ffsm.tile([1, LQ], F32R, tag="rstd")
            with nc.allow_low_precision(reason="f32r matmul operand"):
                nc.vector.reciprocal(rstd, sd)
            shift = ffsm.tile([1, LQ], F32R, tag="shift")
            nc.vector.tensor_mul(shift, mu, rstd)
            nc.vector.tensor_scalar_mul(shift, shift, -1.0)
            rstd_bc = ps_coef3.tile([P, LQ], F32, tag="coef")
            shift_bc = ps_coef3.tile([P, LQ], F32, tag="coef")
            _mm(nc, rstd_bc, ones_1x128, rstd, True, True)
            _mm(nc, shift_bc, ones_1x128, shift, True, True)
            rb = rstd_bc.unsqueeze(1).to_broadcast(normed2.shape)
            sb = shift_bc.unsqueeze(1).to_broadcast(normed2.shape)
            nc.vector.tensor_mul(normed2, x2, rb)
            nc.vector.tensor_add(normed2, normed2, sb)

        ps_mm3 = tc.alloc_tile_pool(name="ps_mm3", bufs=3, space="PSUM")
        ps_ff2 = tc.alloc_tile_pool(name="ps_ff2", bufs=4, space="PSUM")
        h_t = ht_pool.tile([P, FC, LQ], BF16)
        for f in range(FC):
            ps = ps_mm3.tile([P, LQ], F32, tag="mm")
            for c in range(DC):
                _mm(nc, ps, wf1_sb[:, c, f * P:(f + 1) * P], normed2[:, c, :],
                    c == 0, c == DC - 1)
            nc.scalar.activation(h_t[:, f, :], ps, func=GELU_FUNC,
                                 bias=bf1_sb[:, f:f + 1], scale=1.0)
        wf24 = wf23.rearrange("p c (g n) -> p c g n", g=2)  # [128,32,2,512]
        for g in range(2):
            accs = [ps_ff2.tile([P, LQ], F32, tag="ff2acc",
                                name=f"ff2acc_{g}_{i}") for i in range(4)]
            for f in range(FC):
                wf2m = wf2s.tile([P, 4, P], BF16, tag="wf2")
                weng2 = nc.gpsimd if f % 2 == 0 else nc.sync
                weng2.dma_start(wf2m, wf24[:, f, g, :].rearrange(
                    "p (i n) -> p i n", n=P))
                for i in range(4):
                    _mm(nc, accs[i], wf2m[:, i, :], h_t[:, f, :],
                        f == 0, f == FC - 1)
            for i in range(4):
                mcol = g * 4 + i
                osb = outsb_pool.tile([P, LQ], F32, tag="osb")
                nc.vector.tensor_scalar_add(osb, accs[i], bf2_sb[:, mcol:mcol + 1])
                nc.vector.tensor_add(osb, osb, x2[:, mcol, :])
                nc.sync.dma_start(out3[:, mcol, :], osb)
        ps_ff2.release()
        ps_mm3.release()

    wf1_pool.release()
    rs_pool.release()
    podram_pool.release()
    qk_pool.release()
    singles.release()


_CACHED = None


def build():
    global _CACHED
    if _CACHED is None:
        nc = bacc.Bacc("TRN2", target_bir_lowering=False, debug=False,
                       num_devices=NCORES)
        with tile.TileContext(nc) as tc:
            emit(tc)
        nc.compile()
        _CACHED = nc
    return _CACHED


def prep_inputs(inputs):
    """Host-side preprocessing: transposes, slices, LN folds."""
    f = np.float32
    x = np.asarray(inputs["x"], f)
    lcc = np.asarray(inputs["lcc_values"], f)
    w_qkv = np.asarray(inputs["w_qkv"], f)
    b_qkv = np.asarray(inputs["b_qkv"], f)
    w_out = np.asarray(inputs["w_out"], f)
    ln1_g = np.asarray(inputs["ln1_g"], f)
    ln1_b = np.asarray(inputs["ln1_b"], f)
    ln2_g = np.asarray(inputs["ln2_g"], f)
    ln2_b = np.asarray(inputs["ln2_b"], f)
    w_ff1 = np.asarray(inputs["w_ff1"], f)
    b_ff1 = np.asarray(inputs["b_ff1"], f)

    def chunked(b):  # [D] -> [128, DC] with chunk c in column c
        return np.ascontiguousarray(b.reshape(-1, P).T)

    xt = np.ascontiguousarray(x.T)
    sel2_m = np.zeros((P, 2), f)
    sel2_m[0:DH, 0] = 1.0
    sel2_m[DH:P, 1] = 1.0

    shared = {
        "xt": xt,
        "wf1": np.ascontiguousarray(ln2_g[:, None] * w_ff1).astype(ml_dtypes.bfloat16),
        "wf2": np.ascontiguousarray(np.asarray(inputs["w_ff2"], f)).astype(ml_dtypes.bfloat16),
        "bo": chunked(np.asarray(inputs["b_out"], f)),
        "bf1": chunked(b_ff1 + ln2_b @ w_ff1),
        "bf2": chunked(np.asarray(inputs["b_ff2"], f)),
        "explcc": np.ascontiguousarray(np.exp(lcc * (0.5 * LCC)).reshape(KC, P).T),
        "sel2": sel2_m,
        "selb2": np.ascontiguousarray(sel2_m.T),
        "ident": np.eye(P, dtype=f),
        "ones1r": np.ones((1, P), f),
        "ones1f": np.ones((1, P), f),
        "onesc": np.ones((P, 1), f),
    }
    in_maps = []
    for c in range(NCORES):
        m = dict(shared)
        csl = slice(c * P, (c + 1) * P)
        wq_s = ln1_g[:, None] * w_qkv[:, 0:D][:, csl]
        wk_s = ln1_g[:, None] * w_qkv[:, D:2 * D][:, csl]
        wv_s = ln1_g[:, None] * w_qkv[:, 2 * D:3 * D][:, csl]
        m["wq2"] = np.ascontiguousarray(wq_s)
        m["wk2"] = np.ascontiguousarray(wk_s)
        m["wv2"] = np.ascontiguousarray(wv_s)
        for nm, ws, bs in (
            ("cgbq", w_qkv[:, 0:D][:, csl], b_qkv[0:D][csl]),
            ("cgbk", w_qkv[:, D:2 * D][:, csl], b_qkv[D:2 * D][csl]),
            ("cgbv", w_qkv[:, 2 * D:3 * D][:, csl], b_qkv[2 * D:3 * D][csl]),
        ):
            cg = ln1_g @ ws
            cb = bs + ln1_b @ ws
            m[nm] = np.ascontiguousarray(np.stack([cg, cb]).astype(f))
        m["wo2"] = np.ascontiguousarray(w_out[csl, :])
        m["xot"] = np.ascontiguousarray(xt[:, c * LQ:(c + 1) * LQ])
        in_maps.append(m)
    return in_maps


def kernel(**inputs):
    nc = build()
    in_maps = prep_inputs(inputs)
    res = run_bass_kernel_spmd(nc, in_maps, core_ids=list(range(NCORES)))
    out = np.concatenate([res.results[c]["out_t"] for c in range(NCORES)], axis=1)
    return np.ascontiguousarray(out.T).astype(np.float32)
